# revision 7
# baseline (speedup 1.0000x reference)
"""Contrastive loss (NT-Xent style) Trainium2 kernel, 8-core SPMD.

Math: with z_i = normalize(instance_emb.reshape(4096, 512)),
zbag = normalize(bag_emb) [8, 512], z_j = repeat(zbag, 512) and
Z = [z_i; z_j] (8192 rows), the reference computes

  loss = (1/8192) * sum_r [ log(sum_{c != r} exp(2*sim[r,c])) - 2*pos[r] ]

with sim = Z @ Z.T, pos[r] = sim[r, r +- 4096].  Only the
G = z_i @ z_i.T quadrant (4096x4096) needs dense compute:

  denom_i[r] = rowsum(exp(2 G[r,:])) - e^2 + 512 * sum_g exp(2 S1[r,g])
  denom_j[g] = colsum_r(exp(2 S1[r,g])) + 512 * rowsum(exp(2 B[g,:])) - e^2
  pos[r] = pos[bs + r] = S1[r, r // 512]
  loss*8192 = sum_r [log denom_i[r] - 4*S1[r, r//512]] + 512*sum_g log denom_j[g]

where S1 = z_i @ zbag.T and B = zbag @ zbag.T.

Collectives in this rig carry a ~70us fixed penalty (a 2KB AllGather
benches at 84us vs 14us for the same kernel without it), so this
version uses NO collectives: the host replicates bf16 copies of Y
row-major-tiled (for norms) and Y^T (for the gram matmuls) to every
core, cyclically rotated so each core's own 512 rows are local block 0.

Each core computes its 512-row block of G_raw = Y Y^T from the raw
bf16 data (PE is never gated on normalization); 1/||y_r|| 1/||y_c|| is
applied late: per-column via a DVE multiply on the PSUM block with a
broadcast rinv tile, per-row via the activation engine's per-partition
scale operand, fused into exp.  rsqrt uses a linear initial guess
around E||y||^2 = 512 (chi^2_512 concentrates hard) + 2 Newton steps —
vector-only, so the scalar engine keeps a single Exp table loaded all
kernel (ACT table reloads cost 1.5us each).
"""

import os
import numpy as np
import ml_dtypes
from contextlib import ExitStack

import concourse.bass as bass
import concourse.bacc as bacc
import concourse.tile as tile
from concourse import mybir
from concourse import bass_utils
from concourse.masks import make_identity

F32 = mybir.dt.float32
BF16 = mybir.dt.bfloat16

B, N, D = 8, 512, 512
BS = B * N              # 4096 instance rows
NCORES = 8
RPC = BS // NCORES      # 512 rows per core
TPC = RPC // 128        # 4 row-tiles of 128 per core
KC = D // 128           # 4 contraction chunks
NBLK = BS // 512        # 8 column blocks of 512
NPAIR = NBLK // 2       # 4 column block-pairs of 1024
E2 = float(np.exp(2.0))

# linear rsqrt seed around ss ~= 512 (valid +-30%, Newton cleans up)
R0 = float(512.0 ** -0.5)
LIN_A = 1.5 * R0
LIN_B = R0 / (2.0 * 512.0)

LAST_EXEC_TIME_NS = None
_CACHED_NC = None


def _newton2(nc, eng, work, r, ss_slice, width, tag, iters=2):
    """r <- r*(1.5 - 0.5*ss*r^2), `iters` times (on engine `eng`)."""
    a = work.tile([r.shape[0], width], F32, name=f"nta_{tag}")
    for _ in range(iters):
        eng.tensor_mul(a, r, r)
        eng.tensor_mul(a, a, ss_slice)
        eng.tensor_scalar(
            out=a, in0=a, scalar1=-0.5, scalar2=1.5,
            op0=mybir.AluOpType.mult, op1=mybir.AluOpType.add,
        )
        eng.tensor_mul(r, r, a)


def _build_kernel(nc):
    # ybt2: bf16 row-major Y tiled as [128, 32, 512]: [p, g, d] = Y[g*128+p, d]
    ybt2 = nc.dram_tensor("ybt2", [128, BS // 128 * D], BF16, kind="ExternalInput")
    # ytb: bf16 Y^T (columns in local cyclic order), for matmuls
    ytb = nc.dram_tensor("ytb", [D, BS], BF16, kind="ExternalInput")
    bag = nc.dram_tensor("bag", [B, D], F32, kind="ExternalInput")
    onehot = nc.dram_tensor("onehot", [1, B], F32, kind="ExternalInput")
    # out = [partial_c, pad, v_c[0:8] (colsum exp(2 S1own)), rsbg[0:8]]
    out_d = nc.dram_tensor("out", [1, 18], F32, kind="ExternalOutput")

    with ExitStack() as ctx:
        tc = ctx.enter_context(tile.TileContext(nc))

        consts = ctx.enter_context(tc.tile_pool(name="consts", bufs=1))
        work = ctx.enter_context(tc.tile_pool(name="work", bufs=2))
        persist = ctx.enter_context(tc.tile_pool(name="persist", bufs=1))
        dram = ctx.enter_context(tc.tile_pool(name="dram", bufs=1, space="DRAM"))
        ps_main = ctx.enter_context(tc.tile_pool(name="ps_main", bufs=3, space="PSUM"))
        ps_sm = ctx.enter_context(tc.tile_pool(name="ps_sm", bufs=2, space="PSUM"))

        ident = consts.tile([128, 128], F32)
        make_identity(nc, ident)
        ones = consts.tile([128, 1], F32)
        nc.vector.memset(ones, 1.0)
        oh = consts.tile([128, B], F32)
        nc.sync.dma_start(out=oh, in_=onehot.ap().to_broadcast((128, B)))
        bag_t = persist.tile([B, D], F32, name="bag_t")
        nc.sync.dma_start(out=bag_t, in_=bag[:, :])

        # ---- bulk input DMAs, wave-scheduled ----
        ytk = [persist.tile([128, BS], BF16, name=f"ytk_{k}") for k in range(KC)]
        yb = persist.tile([128, BS // 128, D], BF16, name="yb")

        def ytb_wave(cp):  # column pair cp: 8 pieces [64, 1024] on sync
            c0 = cp * 1024
            for k in range(KC):
                for h in range(2):
                    nc.sync.dma_start(
                        out=ytk[k][h * 64:(h + 1) * 64, c0:c0 + 1024],
                        in_=ytb[k * 128 + h * 64: k * 128 + (h + 1) * 64,
                                c0:c0 + 1024],
                    )

        def ybf_wave(j):  # row pair j: 8 pieces [16, 4096] on gpsimd
            g0 = j * 8
            for h in range(8):
                nc.gpsimd.dma_start(
                    out=yb[h * 16:(h + 1) * 16, g0:g0 + 8, :],
                    in_=ybt2[h * 16:(h + 1) * 16,
                             g0 * 512:(g0 + 8) * 512].rearrange(
                        "p (g d) -> p g d", d=512
                    ),
                )

        for w in range(4):
            ytb_wave(w)
            ybf_wave(w)

        # ---- bag chain: sumsq -> linear rsqrt + newton -> zbag -> zbagT ----
        sqb = work.tile([B, D], F32, name="sq_bag")
        nc.vector.tensor_mul(sqb, bag_t, bag_t)
        ssb = persist.tile([B, 1], F32, name="ss_bag")
        nc.vector.reduce_sum(ssb, sqb, axis=mybir.AxisListType.X)
        rb = work.tile([B, 1], F32, name="r_bag")
        nc.vector.tensor_scalar(
            out=rb, in0=ssb, scalar1=-LIN_B, scalar2=LIN_A,
            op0=mybir.AluOpType.mult, op1=mybir.AluOpType.add,
        )
        _newton2(nc, nc.vector, work, rb, ssb, 1, "bag", iters=3)
        zbag = persist.tile([B, D], F32, name="zbag")
        nc.vector.tensor_scalar_mul(zbag, bag_t, rb[:, 0:1])
        zbagT = persist.tile([128, KC, B], BF16, name="zbagT")
        for k in range(KC):
            ptr = ps_sm.tile([128, B], F32, tag="sm", name="ptr_bag")
            nc.tensor.transpose(ptr, zbag[:, k * 128:(k + 1) * 128], ident[:B, :B])
            nc.vector.tensor_copy(zbagT[:, k, :], ptr)

        # ---- norm state ----
        ss = persist.tile([128, NBLK * TPC], F32, name="ss")
        rinv = persist.tile([128, NBLK * TPC], F32, name="rinv")
        rbc = [persist.tile([128, 1024], F32, name=f"rbc_{j}") for j in range(NPAIR)]
        rbd = [dram.tile([1, 1024], F32, name=f"rbd_{j}") for j in range(NPAIR)]

        def rinv_pair(j):
            """sumsq + rsqrt for blocks 2j,2j+1; broadcast tile rbc[j].

            vector: square+reduce+newton; PE: transpose; DMA bounce via
            vector-issued trigger, broadcast via scalar-issued trigger."""
            c0, c1 = 8 * j, 8 * j + 8
            nc.vector.tensor_mul(yb[:, c0:c1, :], yb[:, c0:c1, :], yb[:, c0:c1, :])
            nc.vector.reduce_sum(ss[:, c0:c1], yb[:, c0:c1, :],
                                 axis=mybir.AxisListType.X)
            nc.vector.tensor_scalar(
                out=rinv[:, c0:c1], in0=ss[:, c0:c1], scalar1=-LIN_B, scalar2=LIN_A,
                op0=mybir.AluOpType.mult, op1=mybir.AluOpType.add,
            )
            _newton2(nc, nc.vector, work, rinv[:, c0:c1], ss[:, c0:c1], 8, f"p{j}")
            ptr = ps_sm.tile([8, 128], F32, tag="sm", name=f"ptr_rinv{j}")
            nc.tensor.transpose(ptr, rinv[:, c0:c1], ident)
            rT = work.tile([8, 128], F32, name=f"rT_{j}")
            nc.vector.tensor_copy(rT, ptr)
            nc.scalar.dma_start(
                out=rbd[j].rearrange("1 (t p) -> t p", t=8), in_=rT
            )
            nc.scalar.dma_start(
                out=rbc[j], in_=rbd[j].to_broadcast((128, 1024))
            )

        rinv_pair(0)
        r2own = persist.tile([128, TPC], F32, name="r2own")
        nc.vector.tensor_scalar(
            out=r2own, in0=rinv[:, 0:TPC], scalar1=2.0, scalar2=None,
            op0=mybir.AluOpType.mult,
        )

        # ---- S1 own rows + positives (raw lhsT, fixed up in the exp) ----
        s1rs = persist.tile([128, TPC], F32, name="s1rs")
        pos = persist.tile([128, TPC], F32, name="pos")
        es1 = persist.tile([128, TPC, B], F32, name="es1")
        s1sc = persist.tile([128, B], F32, name="s1sc")
        for t in range(TPC):
            pm = ps_sm.tile([128, B], F32, tag="sm", name="ps_s1own")
            for k in range(KC):
                nc.tensor.matmul(
                    pm,
                    lhsT=ytk[k][:, t * 128:(t + 1) * 128],
                    rhs=zbagT[:, k, :],
                    start=(k == 0),
                    stop=(k == KC - 1),
                )
            nc.scalar.activation(
                es1[:, t, :], pm, mybir.ActivationFunctionType.Exp,
                scale=r2own[:, t:t + 1], accum_out=s1rs[:, t:t + 1],
            )
            nc.vector.tensor_mul(s1sc, pm, oh)
            nc.vector.reduce_sum(pos[:, t:t + 1], s1sc, axis=mybir.AxisListType.X)
        nc.vector.tensor_mul(pos, pos, rinv[:, 0:TPC])

        # ---- Bgram ----
        pbg = ps_sm.tile([B, B], F32, tag="sm", name="ps_bgram")
        for k in range(KC):
            nc.tensor.matmul(
                pbg, lhsT=zbagT[:, k, :], rhs=zbagT[:, k, :],
                start=(k == 0), stop=(k == KC - 1),
            )
        ebg = persist.tile([B, B], F32, name="exp_bgram")
        rsbg = persist.tile([B, 1], F32, name="rs_bgram")
        nc.scalar.activation(
            ebg, pbg, mybir.ActivationFunctionType.Exp, scale=2.0, accum_out=rsbg
        )

        # ---- main loop: raw gram row-tile x col block-pair ----
        rs = persist.tile([128, TPC, NPAIR], F32, name="rs")
        for bb in range(NPAIR):
            for t in range(TPC):
                pm = ps_main.tile([128, 1024], F32, name="ps_g")
                for half in range(2):
                    blk = 2 * bb + half
                    for k in range(KC):
                        nc.tensor.matmul(
                            pm[:, half * 512:(half + 1) * 512],
                            lhsT=ytk[k][:, t * 128:(t + 1) * 128],
                            rhs=ytk[k][:, blk * 512:(blk + 1) * 512],
                            start=(k == 0),
                            stop=(k == KC - 1),
                        )
                nc.vector.tensor_mul(pm, pm, rbc[bb])
                nc.scalar.activation(
                    pm, pm, mybir.ActivationFunctionType.Exp,
                    scale=r2own[:, t:t + 1], accum_out=rs[:, t, bb:bb + 1],
                )
            if bb < NPAIR - 1:
                rinv_pair(bb + 1)

        # ---- denominators + logs ----
        rsum = persist.tile([128, TPC], F32, name="rsum")
        nc.vector.reduce_sum(rsum, rs, axis=mybir.AxisListType.X)
        di = persist.tile([128, TPC], F32, name="di")
        nc.vector.tensor_scalar(
            out=di, in0=s1rs, scalar1=512.0, scalar2=-E2,
            op0=mybir.AluOpType.mult, op1=mybir.AluOpType.add,
        )
        nc.vector.tensor_add(di, di, rsum)
        ldi = persist.tile([128, TPC], F32, name="ldi")
        nc.scalar.activation(ldi, di, mybir.ActivationFunctionType.Ln)

        # colsum over own rows of exp(2 S1own): v[1, g]
        pv = ps_sm.tile([1, B], F32, tag="sm", name="ps_v")
        for t in range(TPC):
            nc.tensor.matmul(
                pv, lhsT=ones, rhs=es1[:, t, :],
                start=(t == 0), stop=(t == TPC - 1),
            )
        vrow = persist.tile([1, B], F32, name="vrow")
        nc.vector.tensor_copy(vrow, pv)

        prb = ps_sm.tile([1, B], F32, tag="sm", name="ps_rbT")
        nc.tensor.transpose(prb, rsbg, ident[:B, :B])
        rsbgT = persist.tile([1, B], F32, name="rsbgT")
        nc.vector.tensor_copy(rsbgT, prb)

        # ---- final combine: fin = sum_t ldi - 4*sum_t pos per partition ----
        fin = persist.tile([128, 1], F32, name="fin")
        vsum = persist.tile([128, 1], F32, name="vsum")
        nc.vector.reduce_sum(vsum, ldi, axis=mybir.AxisListType.X)
        posr = persist.tile([128, 1], F32, name="posr")
        nc.vector.reduce_sum(posr, pos, axis=mybir.AxisListType.X)
        nc.vector.tensor_scalar(
            out=posr, in0=posr, scalar1=-4.0, scalar2=None,
            op0=mybir.AluOpType.mult,
        )
        nc.vector.tensor_add(fin, vsum, posr)

        pfin = ps_sm.tile([1, 1], F32, tag="sm", name="ps_fin")
        nc.tensor.matmul(pfin, lhsT=ones, rhs=fin, start=True, stop=True)
        outt = persist.tile([1, 18], F32, name="outt")
        nc.vector.memset(outt, 0.0)
        nc.vector.tensor_copy(outt[:, 0:1], pfin)
        nc.vector.tensor_copy(outt[:, 2:10], vrow)
        nc.vector.tensor_copy(outt[:, 10:18], rsbgT)
        nc.sync.dma_start(out=out_d[:, :], in_=outt)

    return nc


def _get_nc():
    global _CACHED_NC
    if _CACHED_NC is None:
        nc = bacc.Bacc(
            "TRN2", target_bir_lowering=False, debug=False, num_devices=NCORES
        )
        nc = _build_kernel(nc)
        nc.compile()
        _CACHED_NC = nc
    return _CACHED_NC


def kernel(instance_emb: np.ndarray, bag_emb: np.ndarray) -> np.ndarray:
    global LAST_EXEC_TIME_NS
    Y = np.asarray(instance_emb, dtype=np.float32).reshape(BS, D)
    bg = np.ascontiguousarray(np.asarray(bag_emb, dtype=np.float32))

    in_maps = []
    for c in range(NCORES):
        # cyclic rotation: core c's own rows become local block 0
        Yc = np.roll(Y, -c * RPC, axis=0)
        Yb = Yc.astype(ml_dtypes.bfloat16)
        # [p, g, d] -> flat [128, 32*512]
        ybt2_c = np.ascontiguousarray(
            Yb.reshape(BS // 128, 128, D).transpose(1, 0, 2).reshape(128, -1)
        )
        ytb_c = np.ascontiguousarray(Yc.T.astype(ml_dtypes.bfloat16))
        ohv = np.zeros((1, B), np.float32)
        ohv[0, c] = 1.0
        in_maps.append({"ybt2": ybt2_c, "ytb": ytb_c, "bag": bg, "onehot": ohv})

    nc = _get_nc()
    trace = os.environ.get("CL_KERNEL_TRACE", "0") == "1"
    tmpdir = os.environ.get("CL_KERNEL_TRACE_DIR") or None
    if os.environ.get("CL_KERNEL_WARMUP", "0") == "1":
        bass_utils.run_bass_kernel_spmd(
            nc, in_maps, core_ids=list(range(NCORES)), trace=False
        )
    res = bass_utils.run_bass_kernel_spmd(
        nc, in_maps, core_ids=list(range(NCORES)), trace=trace, tmpdir=tmpdir
    )
    LAST_EXEC_TIME_NS = res.exec_time_ns

    return _assemble([res.results[c]["out"] for c in range(NCORES)])


def _assemble(outs) -> np.ndarray:
    """Combine per-core outputs [1,18] into the scalar loss.

    out row = [partial_c, pad, v_c[0:8], rsbg[0:8]] where
    v_c[g] = sum over core c's rows of exp(2 S1[r,g]) and
    rsbg[g] = sum_g' exp(2 Bgram[g,g']).
    denom_j[g] = sum_c v_c[g] + 512*rsbg[g] - e^2 (identical on all cores).
    """
    total = 0.0
    vsum = np.zeros(B, np.float64)
    for c in range(NCORES):
        o = np.asarray(outs[c], np.float64).reshape(-1)
        total += o[0]
        vsum += o[2:10]
    rsbg = np.asarray(outs[0], np.float64).reshape(-1)[10:18]
    denom_j = vsum + 512.0 * rsbg - E2
    lj = 512.0 * float(np.sum(np.log(denom_j)))
    return np.float32((total + lj) / (2 * BS))


# revision 11
# speedup vs baseline: 1.5914x; 1.5914x over previous
"""Contrastive loss (NT-Xent style) Trainium2 kernel, 8-core SPMD, fp8.

Math: with z_i = normalize(instance_emb.reshape(4096, 512)),
zbag = normalize(bag_emb) [8, 512], Z = [z_i; repeat(zbag, 512)], the
reference loss reduces to (see _assemble):

  denom_i[r] = rowsum(exp(2 G[r,:])) - e^2 + 512 * sum_g exp(2 S1[r,g])
  denom_j[g] = colsum_r(exp(2 S1[r,g])) + 512 * rowsum(exp(2 B[g,:])) - e^2
  loss*8192 = sum_r [log denom_i[r] - 4*S1[r, r//512]] + 512*sum_g log denom_j[g]

with G = z_i z_i^T, S1 = z_i zbag^T, B = zbag zbag^T.

No collectives (a 2KB AllGather costs ~70us fixed in this rig): the
host replicates fp8(16*Y^T) to every core, cyclically rotated so each
core's own 512 rows are local block 0.  Per core:

- G block [own 512 x 4096] in fp8 DoubleRow matmuls (K=256/instr),
  raw y16 values; normalization applied as:
  pair 0 (cols 0:1024): DVE multiply of the PSUM block by broadcast
  rb_c = 1/(16||y||), then exp with per-partition scale 2*rb_own;
  pairs 1-3: rhs pre-scaled once to fp8 z16 = 16*z, exp scale rb_own/8.
- sumsq for the norms comes from the transposed fp8 tiles: DVE squares
  (bf16) + PE ones-matmul partition-reduction -> [8,512] PSUM tiles,
  bounced through DRAM into a [128, 32] per-partition layout.
- rsqrt = linear seed around E||16y||^2 = 131072 + 2 Newton steps,
  vector-only, so the scalar engine keeps its Exp table loaded
  (ACT table reloads cost 1.5us each).
Output: per-core scalar partials; host sums O(8) values.
"""

import os
import numpy as np
import ml_dtypes
from contextlib import ExitStack

import concourse.bass as bass
import concourse.bacc as bacc
import concourse.tile as tile
from concourse import mybir
from concourse import bass_utils
from concourse.masks import make_identity

F32 = mybir.dt.float32
BF16 = mybir.dt.bfloat16
F8 = mybir.dt.float8e4
DR = mybir.MatmulPerfMode.DoubleRow

B, N, D = 8, 512, 512
BS = B * N
NCORES = 8
RPC = BS // NCORES      # 512 rows per core
TPC = RPC // 128        # 4 row-tiles per core
NBLK = BS // 512        # 8 column blocks
NPAIR = NBLK // 2       # 4 column block-pairs
E2 = float(np.exp(2.0))

# linear rsqrt seed around ss8 = ||16 y||^2 ~= 256*512
SS0 = 256.0 * 512.0
R0 = SS0 ** -0.5
LIN_A = 1.5 * R0
LIN_B = R0 / (2.0 * SS0)

LAST_EXEC_TIME_NS = None
_CACHED_NC = None


def _newton(nc, work, r, ss_slice, width, tag, iters=2):
    a = work.tile([128, width], F32, name=f"nta_{tag}")
    for _ in range(iters):
        nc.vector.tensor_mul(a, r, r)
        nc.vector.tensor_mul(a, a, ss_slice)
        nc.vector.tensor_scalar(
            out=a, in0=a, scalar1=-0.5, scalar2=1.5,
            op0=mybir.AluOpType.mult, op1=mybir.AluOpType.add,
        )
        nc.vector.tensor_mul(r, r, a)


def _build_kernel(nc):
    # fp8(16*Y^T), columns in local cyclic order
    ytb = nc.dram_tensor("ytb", [D, BS], F8, kind="ExternalInput")
    bag = nc.dram_tensor("bag", [B, D], F32, kind="ExternalInput")
    onehot = nc.dram_tensor("onehot", [1, B], F32, kind="ExternalInput")
    out_d = nc.dram_tensor("out", [1, 18], F32, kind="ExternalOutput")

    with ExitStack() as ctx:
        tc = ctx.enter_context(tile.TileContext(nc))

        consts = ctx.enter_context(tc.tile_pool(name="consts", bufs=1))
        work = ctx.enter_context(tc.tile_pool(name="work", bufs=2))
        persist = ctx.enter_context(tc.tile_pool(name="persist", bufs=1))
        dram = ctx.enter_context(tc.tile_pool(name="dram", bufs=1, space="DRAM"))
        ps_main = ctx.enter_context(tc.tile_pool(name="ps_main", bufs=3, space="PSUM"))
        ps_sm = ctx.enter_context(tc.tile_pool(name="ps_sm", bufs=2, space="PSUM"))

        ident = consts.tile([128, 128], F32)
        make_identity(nc, ident)
        ones = consts.tile([128, 1], F32)
        nc.vector.memset(ones, 1.0)
        ones8 = consts.tile([128, 8], BF16)
        nc.vector.memset(ones8, 1.0)
        oh = consts.tile([128, B], F32)
        nc.sync.dma_start(out=oh, in_=onehot.ap().to_broadcast((128, B)))
        bag_t = persist.tile([B, D], F32, name="bag_t")
        nc.sync.dma_start(out=bag_t, in_=bag[:, :])

        # ---- input DMA: yt8[kk] = [128(p), 2(j), 4096(c)], d = 256kk+128j+p
        yt8 = [persist.tile([128, 2, BS], F8, name=f"yt8_{kk}") for kk in range(2)]
        for cq in range(4):
            c0 = cq * 1024
            for kk in range(2):
                for j in range(2):
                    for rh in range(2):
                        nc.sync.dma_start(
                            out=yt8[kk][rh * 64:(rh + 1) * 64, j, c0:c0 + 1024],
                            in_=ytb[256 * kk + 128 * j + 64 * rh:
                                    256 * kk + 128 * j + 64 * (rh + 1),
                                    c0:c0 + 1024],
                        )

        # ---- bag chain (fp32, tiny): normalize, scale by 16, to fp8 ----
        sqb = work.tile([B, D], F32, name="sq_bag")
        nc.vector.tensor_mul(sqb, bag_t, bag_t)
        ssb = persist.tile([B, 1], F32, name="ss_bag")
        nc.vector.reduce_sum(ssb, sqb, axis=mybir.AxisListType.X)
        rbg = work.tile([B, 1], F32, name="r_bag")
        # seed for ss ~ 512 (bag rows are unscaled)
        nc.vector.tensor_scalar(
            out=rbg, in0=ssb, scalar1=-(512.0 ** -0.5) / 1024.0,
            scalar2=1.5 * 512.0 ** -0.5,
            op0=mybir.AluOpType.mult, op1=mybir.AluOpType.add,
        )
        a = work.tile([B, 1], F32, name="nta_bag")
        for _ in range(3):
            nc.vector.tensor_mul(a, rbg, rbg)
            nc.vector.tensor_mul(a, a, ssb)
            nc.vector.tensor_scalar(
                out=a, in0=a, scalar1=-0.5, scalar2=1.5,
                op0=mybir.AluOpType.mult, op1=mybir.AluOpType.add,
            )
            nc.vector.tensor_mul(rbg, rbg, a)
        nc.vector.tensor_scalar(
            out=rbg, in0=rbg, scalar1=16.0, scalar2=None, op0=mybir.AluOpType.mult
        )
        zbag16 = persist.tile([B, D], F32, name="zbag16")
        nc.vector.tensor_scalar_mul(zbag16, bag_t, rbg[:, 0:1])
        # zbagT8[kk] = [128, 2, 16] fp8 (B padded to 16 for the DoubleRow
        # 16-byte Ko-stride ISA rule; pad columns are zero)
        zbagT8 = [persist.tile([128, 2, 16], F8, name=f"zbagT8_{kk}") for kk in range(2)]
        for kk in range(2):
            nc.vector.memset(zbagT8[kk], 0.0)
            for j in range(2):
                ptr = ps_sm.tile([128, B], F32, tag="sm", name="ptr_bag")
                d0 = 256 * kk + 128 * j
                nc.tensor.transpose(ptr, zbag16[:, d0:d0 + 128], ident[:B, :B])
                nc.vector.tensor_copy(zbagT8[kk][:, j, 0:B], ptr)

        # ---- S1 own rows (raw lhsT); stash raw psum in SBUF ----
        s1rs = persist.tile([128, TPC], F32, name="s1rs")
        pos = persist.tile([128, TPC], F32, name="pos")
        es1 = persist.tile([128, TPC, B], F32, name="es1")
        s1sc = persist.tile([128, B], F32, name="s1sc")
        s1raw = persist.tile([128, TPC, B], F32, name="s1raw")
        for t in range(TPC):
            pm = ps_sm.tile([128, 16], F32, tag="sm", name="ps_s1")
            for kk in range(2):
                nc.tensor.matmul(
                    pm,
                    lhsT=yt8[kk][:, :, t * 128:(t + 1) * 128],
                    rhs=zbagT8[kk],
                    start=(kk == 0), stop=(kk == 1),
                    perf_mode=DR,
                )
            nc.vector.tensor_copy(s1raw[:, t, :], pm[:, 0:B])

        # ---- sumsq via squares + PE ones-matmul partition reduce ----
        sq16 = [persist.tile([128, 2, BS], BF16, name=f"sq16_{kk}") for kk in range(2)]
        ssd = [dram.tile([1, 1024], F32, name=f"ssd_{j}") for j in range(NPAIR)]
        ssrow = [persist.tile([8, 1024], F32, name=f"ssrow_{j}") for j in range(NPAIR)]
        ss = persist.tile([128, NBLK * TPC], F32, name="ss")
        rb = persist.tile([128, NBLK * TPC], F32, name="rb")
        rbc = [persist.tile([128, 1024], F32, name=f"rbc_{j}") for j in range(NPAIR)]
        rbd = [dram.tile([1, 1024], F32, name=f"rbd_{j}") for j in range(NPAIR)]

        def sumsq_quarter(cq):
            """squares of column quarter cq (= pair cq), ones-matmul
            partition-reduce into ssrow[cq], bounce to DRAM."""
            c0 = cq * 1024
            for kk in range(2):
                for j in range(2):
                    nc.vector.tensor_mul(
                        sq16[kk][:, j, c0:c0 + 1024],
                        yt8[kk][:, j, c0:c0 + 1024],
                        yt8[kk][:, j, c0:c0 + 1024],
                    )
            for h in range(2):
                cb = cq * 2 + h
                pss = ps_sm.tile([8, 512], F32, tag="sm", name=f"ps_ss{cb}")
                first = True
                for kk in range(2):
                    for j in range(2):
                        nc.tensor.matmul(
                            pss, lhsT=ones8,
                            rhs=sq16[kk][:, j, cb * 512:(cb + 1) * 512],
                            start=first, stop=(kk == 1 and j == 1),
                        )
                        first = False
                nc.vector.tensor_copy(ssrow[cq][:, h * 512:(h + 1) * 512], pss)
            nc.scalar.dma_start(out=ssd[cq], in_=ssrow[cq][0:1, :])

        def rsqrt_pair(pj, prescale_scale):
            """rb for pair pj from ssd[pj]; broadcast tile rbc[pj]
            (times 16 for prescale pairs: zt8 = y16*16*rb = 16*z)."""
            c0 = pj * 8
            nc.scalar.dma_start(
                out=ss[:, c0:c0 + 8],
                in_=ssd[pj].rearrange("1 (m p) -> p m", p=128),
            )
            nc.vector.tensor_scalar(
                out=rb[:, c0:c0 + 8], in0=ss[:, c0:c0 + 8],
                scalar1=-LIN_B, scalar2=LIN_A,
                op0=mybir.AluOpType.mult, op1=mybir.AluOpType.add,
            )
            _newton(nc, work, rb[:, c0:c0 + 8], ss[:, c0:c0 + 8], 8, f"p{pj}")
            ptr = ps_sm.tile([8, 128], F32, tag="sm", name=f"ptr_rb{pj}")
            nc.tensor.transpose(ptr, rb[:, c0:c0 + 8], ident)
            rT = work.tile([8, 128], F32, name=f"rT_{pj}")
            nc.vector.tensor_scalar(
                out=rT, in0=ptr, scalar1=prescale_scale, scalar2=None,
                op0=mybir.AluOpType.mult,
            )
            nc.scalar.dma_start(
                out=rbd[pj].rearrange("1 (t p) -> t p", t=8), in_=rT
            )
            nc.scalar.dma_start(out=rbc[pj], in_=rbd[pj].to_broadcast((128, 1024)))

        sumsq_quarter(0)
        sumsq_quarter(1)
        rsqrt_pair(0, 1.0)
        rsqrt_pair(1, 16.0)

        # scale APs: r2own = 2*rb_own (raw+fixup), rs1own = rb_own/8 (prescaled/S1)
        r2own = persist.tile([128, TPC], F32, name="r2own")
        nc.vector.tensor_scalar(
            out=r2own, in0=rb[:, 0:TPC], scalar1=2.0, scalar2=None,
            op0=mybir.AluOpType.mult,
        )
        rs1own = persist.tile([128, TPC], F32, name="rs1own")
        nc.vector.tensor_scalar(
            out=rs1own, in0=rb[:, 0:TPC], scalar1=0.125, scalar2=None,
            op0=mybir.AluOpType.mult,
        )

        # es1 + positives (s1raw = 256 * y.zbag)
        for t in range(TPC):
            nc.scalar.activation(
                es1[:, t, :], s1raw[:, t, :], mybir.ActivationFunctionType.Exp,
                scale=rs1own[:, t:t + 1], accum_out=s1rs[:, t:t + 1],
            )
            nc.vector.tensor_mul(s1sc, s1raw[:, t, :], oh)
            nc.vector.reduce_sum(pos[:, t:t + 1], s1sc, axis=mybir.AxisListType.X)
        nc.vector.tensor_mul(pos, pos, rb[:, 0:TPC])  # pos = 16*sim, folded later

        # ---- Bgram ----
        pbg = ps_sm.tile([B, B], F32, tag="sm", name="ps_bgram")
        for kk in range(2):
            for j in range(2):
                nc.tensor.matmul(
                    pbg, lhsT=zbagT8[kk][:, j, 0:B], rhs=zbagT8[kk][:, j, 0:B],
                    start=(kk == 0 and j == 0), stop=(kk == 1 and j == 1),
                )
        ebg = persist.tile([B, B], F32, name="exp_bgram")
        rsbg = persist.tile([B, 1], F32, name="rs_bgram")
        nc.scalar.activation(
            ebg, pbg, mybir.ActivationFunctionType.Exp, scale=2.0 / 256.0,
            accum_out=rsbg,
        )

        # prescaled fp8 tiles for pairs 1-3 (cols 1024:4096): 16*z = y16 * 16*rb
        zt8 = [persist.tile([128, 2, 3072], F8, name=f"zt8_{kk}") for kk in range(2)]

        def prescale_pair(pj):  # pj in 1..3
            c0 = pj * 1024
            for kk in range(2):
                for j in range(2):
                    nc.vector.tensor_mul(
                        zt8[kk][:, j, c0 - 1024:c0], yt8[kk][:, j, c0:c0 + 1024],
                        rbc[pj],
                    )

        prescale_pair(1)

        # ---- main loop ----
        rs = persist.tile([128, TPC, NPAIR], F32, name="rs")
        for bb in range(NPAIR):
            for t in range(TPC):
                pm = ps_main.tile([128, 1024], F32, name="ps_g")
                for kk in range(2):
                    for half in range(2):
                        blk = 2 * bb + half
                        if bb == 0:
                            rhs = yt8[kk][:, :, blk * 512:(blk + 1) * 512]
                        else:
                            rhs = zt8[kk][:, :, blk * 512 - 1024:(blk + 1) * 512 - 1024]
                        nc.tensor.matmul(
                            pm[:, half * 512:(half + 1) * 512],
                            lhsT=yt8[kk][:, :, t * 128:(t + 1) * 128],
                            rhs=rhs,
                            start=(kk == 0), stop=(kk == 1),
                            perf_mode=DR,
                        )
                if bb == 0:
                    nc.vector.tensor_mul(pm, pm, rbc[0])
                    sc = r2own
                else:
                    sc = rs1own
                nc.scalar.activation(
                    pm, pm, mybir.ActivationFunctionType.Exp,
                    scale=sc[:, t:t + 1], accum_out=rs[:, t, bb:bb + 1],
                )
            if bb == 0:
                sumsq_quarter(2)
                rsqrt_pair(2, 16.0)
                prescale_pair(2)
            elif bb == 1:
                sumsq_quarter(3)
                rsqrt_pair(3, 16.0)
                prescale_pair(3)

        # ---- denominators + logs ----
        rsum = persist.tile([128, TPC], F32, name="rsum")
        nc.vector.reduce_sum(rsum, rs, axis=mybir.AxisListType.X)
        di = persist.tile([128, TPC], F32, name="di")
        nc.vector.tensor_scalar(
            out=di, in0=s1rs, scalar1=512.0, scalar2=-E2,
            op0=mybir.AluOpType.mult, op1=mybir.AluOpType.add,
        )
        nc.vector.tensor_add(di, di, rsum)
        ldi = persist.tile([128, TPC], F32, name="ldi")
        nc.scalar.activation(ldi, di, mybir.ActivationFunctionType.Ln)

        pv = ps_sm.tile([1, B], F32, tag="sm", name="ps_v")
        for t in range(TPC):
            nc.tensor.matmul(
                pv, lhsT=ones, rhs=es1[:, t, :],
                start=(t == 0), stop=(t == TPC - 1),
            )
        vrow = persist.tile([1, B], F32, name="vrow")
        nc.vector.tensor_copy(vrow, pv)

        prb = ps_sm.tile([1, B], F32, tag="sm", name="ps_rbT")
        nc.tensor.transpose(prb, rsbg, ident[:B, :B])
        rsbgT = persist.tile([1, B], F32, name="rsbgT")
        nc.vector.tensor_copy(rsbgT, prb)

        # fin = sum_t ldi - (4/16)*sum_t pos
        fin = persist.tile([128, 1], F32, name="fin")
        vsum = persist.tile([128, 1], F32, name="vsum")
        nc.vector.reduce_sum(vsum, ldi, axis=mybir.AxisListType.X)
        posr = persist.tile([128, 1], F32, name="posr")
        nc.vector.reduce_sum(posr, pos, axis=mybir.AxisListType.X)
        nc.vector.tensor_scalar(
            out=posr, in0=posr, scalar1=-0.25, scalar2=None,
            op0=mybir.AluOpType.mult,
        )
        nc.vector.tensor_add(fin, vsum, posr)

        pfin = ps_sm.tile([1, 1], F32, tag="sm", name="ps_fin")
        nc.tensor.matmul(pfin, lhsT=ones, rhs=fin, start=True, stop=True)
        outt = persist.tile([1, 18], F32, name="outt")
        nc.vector.memset(outt, 0.0)
        nc.vector.tensor_copy(outt[:, 0:1], pfin)
        nc.vector.tensor_copy(outt[:, 2:10], vrow)
        nc.vector.tensor_copy(outt[:, 10:18], rsbgT)
        nc.sync.dma_start(out=out_d[:, :], in_=outt)

    return nc


def _get_nc():
    global _CACHED_NC
    if _CACHED_NC is None:
        nc = bacc.Bacc(
            "TRN2", target_bir_lowering=False, debug=False, num_devices=NCORES
        )
        nc = _build_kernel(nc)
        nc.compile()
        _CACHED_NC = nc
    return _CACHED_NC


def kernel(instance_emb: np.ndarray, bag_emb: np.ndarray) -> np.ndarray:
    global LAST_EXEC_TIME_NS
    Y = np.asarray(instance_emb, dtype=np.float32).reshape(BS, D)
    bg = np.ascontiguousarray(np.asarray(bag_emb, dtype=np.float32))

    in_maps = []
    for c in range(NCORES):
        Yc = np.roll(Y, -c * RPC, axis=0)
        yt16 = np.clip(Yc.T * 16.0, -240.0, 240.0)
        ytb_c = np.ascontiguousarray(yt16.astype(ml_dtypes.float8_e4m3))
        ohv = np.zeros((1, B), np.float32)
        ohv[0, c] = 1.0
        in_maps.append({"ytb": ytb_c, "bag": bg, "onehot": ohv})

    nc = _get_nc()
    trace = os.environ.get("CL_KERNEL_TRACE", "0") == "1"
    tmpdir = os.environ.get("CL_KERNEL_TRACE_DIR") or None
    if os.environ.get("CL_KERNEL_WARMUP", "0") == "1":
        bass_utils.run_bass_kernel_spmd(
            nc, in_maps, core_ids=list(range(NCORES)), trace=False
        )
    res = bass_utils.run_bass_kernel_spmd(
        nc, in_maps, core_ids=list(range(NCORES)), trace=trace, tmpdir=tmpdir
    )
    LAST_EXEC_TIME_NS = res.exec_time_ns

    return _assemble([res.results[c]["out"] for c in range(NCORES)])


def _assemble(outs) -> np.ndarray:
    """out row = [partial_c, pad, v_c[0:8], rsbg[0:8]];
    denom_j[g] = sum_c v_c[g] + 512*rsbg[g] - e^2."""
    total = 0.0
    vsum = np.zeros(B, np.float64)
    for c in range(NCORES):
        o = np.asarray(outs[c], np.float64).reshape(-1)
        total += o[0]
        vsum += o[2:10]
    rsbg = np.asarray(outs[0], np.float64).reshape(-1)[10:18]
    denom_j = vsum + 512.0 * rsbg - E2
    lj = 512.0 * float(np.sum(np.log(denom_j)))
    return np.float32((total + lj) / (2 * BS))
